# revision 2
# baseline (speedup 1.0000x reference)
"""Trainium2 Bass kernel for the DEN (Mahalanobis distance) layer.

Computes out[b, e] = (x_b - c_e)^T Sigma_e^{-1} (x_b - c_e) for
x [8192, 128], Centroids [128, 1, 128], Sigmas [128, 128, 128].

Strategy (v6: fp16-highbyte fp8 DoubleRow)
------------------------------------------
Wrapped-diagonal decomposition as in v4 (66 coefficient packs), but the 64
off-diagonal product packs are written by the DVE as fp16 (2x-mode rate,
~0.54 ns/elem) scaled by 1/32, and the PE reads their HIGH BYTES through a
stride-2 fp8e5 view (fp16 truncation == e5m2).  Pairs of packs feed fp8
DoubleRow matmuls (2 packs per instruction) with e4m3 coefficients scaled
by 32 * c (c compensates the truncation bias), accumulating into the SAME
f32 PSUM banks as the bf16 linear/x^2 matmuls, so no combine pass is
needed.  Act evicts PSUM with bias=tv.

Measured engine rates (HW probes): DVE tensor ops 0.54 ns/elem (2-byte
APs); bf16 matmul 246 ns / 512 cols; fp8 DR matmul ~300 ns / 512 cols
(2 packs); dual DMA queues (sync + scalar) ~190 GB/s each.

Sharding: data-parallel over batch B across the 8 cores (1024 rows each);
coefficients replicated.
"""

import os
import sys

sys.path.insert(0, "/opt/trn_rl_repo")

import numpy as np
import ml_dtypes

E, B, D = 128, 8192, 128
NCORES = 8
BLOC = B // NCORES          # 1024 batch rows per core
BT = 512                    # matmul free-dim tile (one PSUM bank)
NSLOT = 16                  # rotation slots: 0..7 then 8,16,...,64
ROTVALS = tuple(range(8)) + tuple(range(8, 65, 8))
NWARM = 5
NPAIR = 32                  # 64 off-diag packs -> 32 DoubleRow pairs
CCOMP = 1.06                # e5m2 truncation compensation (host-tuned)

# product groups: (name, in0 slot range [lo,hi), in1 slot).
GROUPS = [
    ("gA", 1, 3, 3),    # j = 2,1
    ("gB", 0, 1, 3),    # j = 3
    ("gC", 0, 4, 7),    # j = 7,6,5,4
    ("gD", 0, 1, 8),    # j = 8
    ("gE", 0, 8, 9),    # j = 16..9
    ("gF", 0, 8, 10),   # j = 24..17
    ("gG", 0, 8, 11),   # j = 32..25
    ("gH", 0, 8, 12),   # j = 40..33
    ("gI", 0, 8, 13),   # j = 48..41
    ("gJ", 0, 8, 14),   # j = 56..49
    ("gK", 0, 8, 15),   # j = 64..57
]

ORDER = []   # diagonal j per off-diag pack position (0..63)
AVAL = []    # row rotation a per pack position
for _name, _lo, _hi, _s1 in GROUPS:
    for _i in range(_hi - _lo):
        ORDER.append(ROTVALS[_s1] - ROTVALS[_lo + _i])
        AVAL.append(ROTVALS[_lo + _i])
assert len(ORDER) == 2 * NPAIR

bf16 = ml_dtypes.bfloat16
f8e4m3 = ml_dtypes.float8_e4m3

_STATE: dict = {}


def _build_module():
    import concourse.bacc as bacc
    import concourse.tile as tile
    import concourse.mybir as mybir
    from contextlib import ExitStack

    nc = bacc.Bacc("TRN2", target_bir_lowering=False, debug=False)

    xr_d = nc.dram_tensor("xrot", [D, NSLOT * BLOC], mybir.dt.bfloat16,
                          kind="ExternalInput")
    cwb_d = nc.dram_tensor("cwb", [D, 2 * E], mybir.dt.bfloat16,
                           kind="ExternalInput")
    cwf_d = nc.dram_tensor("cwf", [D, NPAIR * 2 * E], mybir.dt.float8e4,
                           kind="ExternalInput")
    tv_d = nc.dram_tensor("tv", [E, 1], mybir.dt.float32, kind="ExternalInput")
    out_d = nc.dram_tensor("out", [E, BLOC], mybir.dt.float32,
                           kind="ExternalOutput")

    f32 = mybir.dt.float32
    b16 = mybir.dt.bfloat16
    f16 = mybir.dt.float16
    f8e5 = mybir.dt.float8e5
    Ident = mybir.ActivationFunctionType.Identity
    Alu = mybir.AluOpType
    DR = mybir.MatmulPerfMode.DoubleRow

    with tile.TileContext(nc) as tc, ExitStack() as ctx:
        const_pool = ctx.enter_context(tc.tile_pool(name="const", bufs=1))
        psum_pool = ctx.enter_context(tc.tile_pool(name="acc", bufs=2, space="PSUM"))

        # PE warmup on a GpSimd-memset tile: no DMA dependency, trips the
        # clock gate during the DMA prologue.
        WU = const_pool.tile([D, BT], b16, tag="warm")
        nc.gpsimd.memset(WU[:, :], 0)
        PSW = psum_pool.tile([E, BT], f32, tag="psw", name="psw")
        for _ in range(NWARM):
            nc.tensor.matmul(PSW[:, :], WU[:, 0:E], WU[:, :],
                             start=True, stop=True, skip_group_check=True)

        ROTS = const_pool.tile([D, NSLOT * BLOC], b16, tag="rots")
        R3 = ROTS[:, :].rearrange("p (s b) -> p s b", s=NSLOT)
        H = const_pool.tile([D, 64 * BLOC], f16, tag="H")
        H3 = H[:, :].rearrange("p (k b) -> p k b", k=64)
        X2 = const_pool.tile([D, BLOC], b16, tag="x2")
        CWB = const_pool.tile([D, 2 * E], b16, tag="cwb")
        CWF = const_pool.tile([D, NPAIR * 2 * E], mybir.dt.float8e4, tag="cwf")
        CWF4 = CWF[:, :].rearrange("p (t two e) -> p t two e", t=NPAIR, two=2)
        TV = const_pool.tile([E, 1], f32, tag="tv")
        OT = const_pool.tile([E, BLOC], f32, tag="ot")

        # stride-2 fp8e5 view of H high bytes: [D, pack, col, (lo,hi)] -> hi
        He5 = H[:, :].bitcast(f8e5)
        HV = He5.rearrange("p (k b s) -> p k b s", k=64, s=2)[:, :, :, 1:2]

        def dma_rots(eng, lo, hi):
            eng.dma_start(ROTS[:, lo * BLOC:hi * BLOC],
                          xr_d.ap()[:, lo * BLOC:hi * BLOC])

        # DMA schedule: 2 HW queues.  sync: a-slots 0..7 + small consts;
        # scalar: fp8 coefficient chunks + b-slots 8..15.
        nc.sync.dma_start(CWB[:, :], cwb_d.ap())
        nc.sync.dma_start(TV[:, :], tv_d.ap())
        dma_rots(nc.sync, 0, 2)
        dma_rots(nc.sync, 2, 4)
        dma_rots(nc.sync, 4, 6)
        dma_rots(nc.sync, 6, 8)
        c0 = 8 * 2 * E          # pairs 0..7
        c1 = 16 * 2 * E         # pairs 8..15
        nc.scalar.dma_start(CWF[:, 0:c0], cwf_d.ap()[:, 0:c0])
        dma_rots(nc.scalar, 8, 10)
        nc.scalar.dma_start(CWF[:, c0:c1], cwf_d.ap()[:, c0:c1])
        dma_rots(nc.scalar, 10, 12)
        nc.scalar.dma_start(CWF[:, c1:], cwf_d.ap()[:, c1:])
        dma_rots(nc.scalar, 12, 14)
        dma_rots(nc.scalar, 14, 16)

        # x^2 on Act from slot 0
        nc.scalar.square(X2[:, :], ROTS[:, 0:BLOC])

        PS = psum_pool.tile([E, BLOC], f32, tag="ps", name="ps")

        # bf16 chain: linear (rhs = x) starts both banks; then x^2
        for bt in range(2):
            nc.tensor.matmul(PS[:, bt * BT:(bt + 1) * BT], CWB[:, 0:E],
                             ROTS[:, bt * BT:(bt + 1) * BT],
                             start=True, stop=False)
        for bt in range(2):
            nc.tensor.matmul(PS[:, bt * BT:(bt + 1) * BT], CWB[:, E:2 * E],
                             X2[:, bt * BT:(bt + 1) * BT],
                             start=False, stop=False)

        # DVE products (fp16, scaled 1/32) + fp8 DR matmuls chained per pair
        pos = 0

        def emit_pairs_until(limit):
            nonlocal pos
            while pos + 2 <= limit:
                t = pos // 2
                for h in range(2):
                    rhs = HV[:, 2 * t:2 * t + 2,
                             h * BT:(h + 1) * BT, :].rearrange(
                                 "p two b one -> p two (b one)")
                    nc.tensor.matmul(PS[:, h * BT:(h + 1) * BT],
                                     CWF4[:, t], rhs,
                                     start=False, stop=(t == NPAIR - 1),
                                     perf_mode=DR)
                pos += 2

        gpos = 0
        for name, lo, hi, s1 in GROUPS:
            w = hi - lo
            nc.vector.scalar_tensor_tensor(
                H3[:, gpos:gpos + w, :],
                R3[:, lo:hi, :],
                1.0 / 32.0,
                R3[:, s1:s1 + 1, :].broadcast_to((D, w, BLOC)),
                Alu.mult, Alu.mult,
            )
            gpos += w
            emit_pairs_until(gpos)
        assert pos == 64 and gpos == 64

        # eviction with bias=tv, then output DMA on both queues
        nc.scalar.activation(OT[:, 0:BT], PS[:, 0:BT], Ident, bias=TV[:, 0:1])
        nc.sync.dma_start(out_d.ap()[:, 0:BT], OT[:, 0:BT])
        nc.scalar.activation(OT[:, BT:BLOC], PS[:, BT:BLOC], Ident,
                             bias=TV[:, 0:1])
        nc.scalar.dma_start(out_d.ap()[:, BT:BLOC], OT[:, BT:BLOC])

    nc.compile()
    return nc


def _host_precompute(Centroids: np.ndarray, Sigmas: np.ndarray):
    Sinv = np.linalg.inv(Sigmas.astype(np.float64))
    A = 0.5 * (Sinv + np.swapaxes(Sinv, 1, 2))          # [E, D, D]
    c = Centroids[:, 0, :].astype(np.float64)           # [E, D]
    Ac = np.einsum("edk,ek->ed", A, c)

    idx = np.arange(D)
    cwb = np.zeros((D, 2, E), np.float32)
    cwb[:, 0, :] = (-2.0 * Ac.T)
    cwb[:, 1, :] = A[:, idx, idx].T
    cwb_host = np.ascontiguousarray(cwb.reshape(D, 2 * E)).astype(bf16)

    cwf = np.zeros((D, 64, E), np.float32)
    for k, (j, a) in enumerate(zip(ORDER, AVAL)):
        s = 2.0 if 1 <= j <= 63 else 1.0
        cwf[:, k, :] = (s * 32.0 * CCOMP) * A[:, (idx + a) % D,
                                              (idx + a + j) % D].T
    cwf_host = np.ascontiguousarray(cwf.reshape(D, NPAIR * 2 * E)).astype(f8e4m3)

    tv_host = np.ascontiguousarray(
        np.einsum("ed,ed->e", Ac, c).astype(np.float32)[:, None])
    return cwb_host, cwf_host, tv_host


def _get_nc():
    if "nc" not in _STATE:
        os.environ.setdefault("JAX_COMPILATION_CACHE_DIR", "/root/.jax_cache")
        _STATE["nc"] = _build_module()
    return _STATE["nc"]


def _make_in_maps(x, Centroids, Sigmas):
    cwb_host, cwf_host, tv_host = _host_precompute(
        np.asarray(Centroids, np.float32), np.asarray(Sigmas, np.float32)
    )
    xT = np.ascontiguousarray(np.asarray(x, np.float32).T).astype(bf16)  # [D, B]
    in_maps = []
    for cidx in range(NCORES):
        xTs = np.ascontiguousarray(xT[:, cidx * BLOC:(cidx + 1) * BLOC])
        xrot = np.concatenate(
            [np.roll(xTs, -r, axis=0) for r in ROTVALS], axis=1)   # [D, 16*BLOC]
        in_maps.append({
            "xrot": np.ascontiguousarray(xrot),
            "cwb": cwb_host,
            "cwf": cwf_host,
            "tv": tv_host,
        })
    return in_maps


def _run_device(in_maps, trace=False):
    from concourse import bass_utils

    nc = _get_nc()
    return bass_utils.run_bass_kernel_spmd(
        nc, in_maps, core_ids=list(range(NCORES)), trace=trace
    )


def kernel(x, Centroids, Sigmas):
    in_maps = _make_in_maps(x, Centroids, Sigmas)
    res = _run_device(in_maps)
    outT = np.concatenate([res.results[c]["out"] for c in range(NCORES)], axis=1)
    return np.ascontiguousarray(outT.T).astype(np.float32)


# revision 4
# speedup vs baseline: 1.4977x; 1.4977x over previous
"""Trainium2 Bass kernel for the DEN (Mahalanobis distance) layer.

Computes out[b, e] = (x_b - c_e)^T Sigma_e^{-1} (x_b - c_e) for
x [8192, 128], Centroids [128, 1, 128], Sigmas [128, 128, 128].

Strategy (v6: fp16-highbyte fp8 DoubleRow)
------------------------------------------
Wrapped-diagonal decomposition as in v4 (66 coefficient packs), but the 64
off-diagonal product packs are written by the DVE as fp16 (2x-mode rate,
~0.54 ns/elem) scaled by 1/32, and the PE reads their HIGH BYTES through a
stride-2 fp8e5 view (fp16 truncation == e5m2).  Pairs of packs feed fp8
DoubleRow matmuls (2 packs per instruction) with e4m3 coefficients scaled
by 32 * c (c compensates the truncation bias), accumulating into the SAME
f32 PSUM banks as the bf16 linear/x^2 matmuls, so no combine pass is
needed.  Act evicts PSUM with bias=tv.

Measured engine rates (HW probes): DVE tensor ops 0.54 ns/elem (2-byte
APs); bf16 matmul 246 ns / 512 cols; fp8 DR matmul ~300 ns / 512 cols
(2 packs); dual DMA queues (sync + scalar) ~190 GB/s each.

Sharding: data-parallel over batch B across the 8 cores (1024 rows each);
coefficients replicated.
"""

import os
import sys

sys.path.insert(0, "/opt/trn_rl_repo")

import numpy as np
import ml_dtypes

E, B, D = 128, 8192, 128
NCORES = 8
BLOC = B // NCORES          # 1024 batch rows per core
BT = 512                    # matmul free-dim tile (one PSUM bank)
NSLOT = 16                  # rotation slots: 0..7 then 8,16,...,64
ROTVALS = tuple(range(8)) + tuple(range(8, 65, 8))
NWARM = 5
NPAIR = 32                  # 64 off-diag packs -> 32 DoubleRow pairs
CCOMP = 1.06                # e5m2 truncation compensation (host-tuned)

# product groups: (name, in0 slot range [lo,hi), in1 slot).
GROUPS = [
    ("gA", 1, 3, 3),    # j = 2,1
    ("gB", 0, 1, 3),    # j = 3
    ("gC", 0, 4, 7),    # j = 7,6,5,4
    ("gD", 0, 1, 8),    # j = 8
    ("gE", 0, 8, 9),    # j = 16..9
    ("gF", 0, 8, 10),   # j = 24..17
    ("gG", 0, 8, 11),   # j = 32..25
    ("gH", 0, 8, 12),   # j = 40..33
    ("gI", 0, 8, 13),   # j = 48..41
    ("gJ", 0, 8, 14),   # j = 56..49
    ("gK", 0, 8, 15),   # j = 64..57
]

ORDER = []   # diagonal j per off-diag pack position (0..63)
AVAL = []    # row rotation a per pack position
for _name, _lo, _hi, _s1 in GROUPS:
    for _i in range(_hi - _lo):
        ORDER.append(ROTVALS[_s1] - ROTVALS[_lo + _i])
        AVAL.append(ROTVALS[_lo + _i])
assert len(ORDER) == 2 * NPAIR

bf16 = ml_dtypes.bfloat16
f8e4m3 = ml_dtypes.float8_e4m3

_STATE: dict = {}


def _build_module():
    import concourse.bacc as bacc
    import concourse.tile as tile
    import concourse.mybir as mybir
    from contextlib import ExitStack

    nc = bacc.Bacc("TRN2", target_bir_lowering=False, debug=False)

    xr_d = nc.dram_tensor("xrot", [D, NSLOT * BLOC], mybir.dt.bfloat16,
                          kind="ExternalInput")
    cwb_d = nc.dram_tensor("cwb", [D, 2 * E], mybir.dt.bfloat16,
                           kind="ExternalInput")
    cwf_d = nc.dram_tensor("cwf", [D, NPAIR * 2 * E], mybir.dt.float8e4,
                           kind="ExternalInput")
    tv_d = nc.dram_tensor("tv", [E, 1], mybir.dt.float32, kind="ExternalInput")
    out_d = nc.dram_tensor("out", [E, BLOC], mybir.dt.float32,
                           kind="ExternalOutput")

    f32 = mybir.dt.float32
    b16 = mybir.dt.bfloat16
    f16 = mybir.dt.float16
    f8e5 = mybir.dt.float8e5
    Ident = mybir.ActivationFunctionType.Identity
    Alu = mybir.AluOpType
    DR = mybir.MatmulPerfMode.DoubleRow

    with tile.TileContext(nc) as tc, ExitStack() as ctx:
        const_pool = ctx.enter_context(tc.tile_pool(name="const", bufs=1))
        psum_pool = ctx.enter_context(tc.tile_pool(name="acc", bufs=2, space="PSUM"))

        # PE warmup on a GpSimd-memset tile: no DMA dependency, trips the
        # clock gate during the DMA prologue.
        WU = const_pool.tile([D, BT], b16, tag="warm")
        nc.gpsimd.memset(WU[:, :], 0)
        PSW = psum_pool.tile([E, BT], f32, tag="psw", name="psw")
        for _ in range(NWARM):
            nc.tensor.matmul(PSW[:, :], WU[:, 0:E], WU[:, :],
                             start=True, stop=True, skip_group_check=True)

        ROTS = const_pool.tile([D, NSLOT * BLOC], b16, tag="rots")
        R3 = ROTS[:, :].rearrange("p (s b) -> p s b", s=NSLOT)
        H = const_pool.tile([D, 64 * BLOC], f16, tag="H")
        H3 = H[:, :].rearrange("p (k b) -> p k b", k=64)
        X2 = const_pool.tile([D, BLOC], b16, tag="x2")
        CWB = const_pool.tile([D, 2 * E], b16, tag="cwb")
        CWF = const_pool.tile([D, NPAIR * 2 * E], mybir.dt.float8e4, tag="cwf")
        CWF4 = CWF[:, :].rearrange("p (t two e) -> p t two e", t=NPAIR, two=2)
        TV = const_pool.tile([E, 1], f32, tag="tv")
        OT = const_pool.tile([E, BLOC], f32, tag="ot")

        # stride-2 fp8e5 view of H high bytes: [D, pack, col, (lo,hi)] -> hi
        He5 = H[:, :].bitcast(f8e5)
        HV = He5.rearrange("p (k b s) -> p k b s", k=64, s=2)[:, :, :, 1:2]

        def dma_rots(eng, lo, hi):
            eng.dma_start(ROTS[:, lo * BLOC:hi * BLOC],
                          xr_d.ap()[:, lo * BLOC:hi * BLOC])

        # DMA schedule: 2 HW queues.  sync: a-slots 0..7 + small consts;
        # scalar: fp8 coefficient chunks + b-slots 8..15.
        nc.sync.dma_start(CWB[:, :], cwb_d.ap())
        nc.sync.dma_start(TV[:, :], tv_d.ap())
        dma_rots(nc.sync, 0, 2)
        dma_rots(nc.sync, 2, 4)
        dma_rots(nc.sync, 4, 6)
        dma_rots(nc.sync, 6, 8)
        c0 = 8 * 2 * E          # pairs 0..7
        c1 = 16 * 2 * E         # pairs 8..15
        nc.scalar.dma_start(CWF[:, 0:c0], cwf_d.ap()[:, 0:c0])
        dma_rots(nc.scalar, 8, 10)
        nc.scalar.dma_start(CWF[:, c0:c1], cwf_d.ap()[:, c0:c1])
        dma_rots(nc.scalar, 10, 12)
        nc.scalar.dma_start(CWF[:, c1:], cwf_d.ap()[:, c1:])
        dma_rots(nc.scalar, 12, 14)
        dma_rots(nc.scalar, 14, 16)

        # x^2 on Act from slot 0
        nc.scalar.square(X2[:, :], ROTS[:, 0:BLOC])

        PS = psum_pool.tile([E, BLOC], f32, tag="ps", name="ps")

        # bf16 chain: linear (rhs = x) starts both banks; then x^2
        for bt in range(2):
            nc.tensor.matmul(PS[:, bt * BT:(bt + 1) * BT], CWB[:, 0:E],
                             ROTS[:, bt * BT:(bt + 1) * BT],
                             start=True, stop=False)
        for bt in range(2):
            nc.tensor.matmul(PS[:, bt * BT:(bt + 1) * BT], CWB[:, E:2 * E],
                             X2[:, bt * BT:(bt + 1) * BT],
                             start=False, stop=False)

        # DVE products (fp16, scaled 1/32) + fp8 DR matmuls chained per pair
        pos = 0

        def emit_pairs_until(limit):
            nonlocal pos
            while pos + 2 <= limit:
                t = pos // 2
                for h in range(2):
                    rhs = HV[:, 2 * t:2 * t + 2,
                             h * BT:(h + 1) * BT, :].rearrange(
                                 "p two b one -> p two (b one)")
                    nc.tensor.matmul(PS[:, h * BT:(h + 1) * BT],
                                     CWF4[:, t], rhs,
                                     start=False, stop=(t == NPAIR - 1),
                                     perf_mode=DR)
                pos += 2

        gpos = 0
        for name, lo, hi, s1 in GROUPS:
            w = hi - lo
            nc.vector.tensor_mul(
                H3[:, gpos:gpos + w, :],
                R3[:, lo:hi, :],
                R3[:, s1:s1 + 1, :].broadcast_to((D, w, BLOC)),
            )
            gpos += w
            emit_pairs_until(gpos)
        assert pos == 64 and gpos == 64

        # eviction with bias=tv, then output DMA on both queues
        nc.scalar.activation(OT[:, 0:BT], PS[:, 0:BT], Ident, bias=TV[:, 0:1])
        nc.sync.dma_start(out_d.ap()[:, 0:BT], OT[:, 0:BT])
        nc.scalar.activation(OT[:, BT:BLOC], PS[:, BT:BLOC], Ident,
                             bias=TV[:, 0:1])
        nc.scalar.dma_start(out_d.ap()[:, BT:BLOC], OT[:, BT:BLOC])

    nc.compile()
    return nc


def _host_precompute(Centroids: np.ndarray, Sigmas: np.ndarray):
    Sinv = np.linalg.inv(Sigmas.astype(np.float64))
    A = 0.5 * (Sinv + np.swapaxes(Sinv, 1, 2))          # [E, D, D]
    c = Centroids[:, 0, :].astype(np.float64)           # [E, D]
    Ac = np.einsum("edk,ek->ed", A, c)

    idx = np.arange(D)
    cwb = np.zeros((D, 2, E), np.float32)
    cwb[:, 0, :] = (-2.0 * 4.0 * Ac.T)          # rhs is x/4
    cwb[:, 1, :] = 16.0 * A[:, idx, idx].T      # rhs is (x/4)^2
    cwb_host = np.ascontiguousarray(cwb.reshape(D, 2 * E)).astype(bf16)

    cwf = np.zeros((D, 64, E), np.float32)
    for k, (j, a) in enumerate(zip(ORDER, AVAL)):
        s = 2.0 if 1 <= j <= 63 else 1.0
        cwf[:, k, :] = (s * 16.0 * CCOMP) * A[:, (idx + a) % D,
                                              (idx + a + j) % D].T
    cwf_host = np.ascontiguousarray(cwf.reshape(D, NPAIR * 2 * E)).astype(f8e4m3)

    tv_host = np.ascontiguousarray(
        np.einsum("ed,ed->e", Ac, c).astype(np.float32)[:, None])
    return cwb_host, cwf_host, tv_host


def _get_nc():
    if "nc" not in _STATE:
        os.environ.setdefault("JAX_COMPILATION_CACHE_DIR", "/root/.jax_cache")
        _STATE["nc"] = _build_module()
    return _STATE["nc"]


def _make_in_maps(x, Centroids, Sigmas):
    cwb_host, cwf_host, tv_host = _host_precompute(
        np.asarray(Centroids, np.float32), np.asarray(Sigmas, np.float32)
    )
    # rotations pre-scaled by 1/4 (exact in bf16): products land at p/16,
    # matching the x16 coefficient scaling -> single-PSUM accumulation
    xT = np.ascontiguousarray(np.asarray(x, np.float32).T * 0.25).astype(bf16)
    in_maps = []
    for cidx in range(NCORES):
        xTs = np.ascontiguousarray(xT[:, cidx * BLOC:(cidx + 1) * BLOC])
        xrot = np.concatenate(
            [np.roll(xTs, -r, axis=0) for r in ROTVALS], axis=1)   # [D, 16*BLOC]
        in_maps.append({
            "xrot": np.ascontiguousarray(xrot),
            "cwb": cwb_host,
            "cwf": cwf_host,
            "tv": tv_host,
        })
    return in_maps


def _run_device(in_maps, trace=False):
    from concourse import bass_utils

    nc = _get_nc()
    return bass_utils.run_bass_kernel_spmd(
        nc, in_maps, core_ids=list(range(NCORES)), trace=trace
    )


def kernel(x, Centroids, Sigmas):
    in_maps = _make_in_maps(x, Centroids, Sigmas)
    res = _run_device(in_maps)
    outT = np.concatenate([res.results[c]["out"] for c in range(NCORES)], axis=1)
    return np.ascontiguousarray(outT.T).astype(np.float32)


# revision 5
# speedup vs baseline: 1.5514x; 1.0359x over previous
"""Trainium2 Bass kernel for the DEN (Mahalanobis distance) layer.

Computes out[b, e] = (x_b - c_e)^T Sigma_e^{-1} (x_b - c_e) for
x [8192, 128], Centroids [128, 1, 128], Sigmas [128, 128, 128].

Strategy (v6: fp16-highbyte fp8 DoubleRow)
------------------------------------------
Wrapped-diagonal decomposition (66 coefficient packs).  The 64 off-diagonal
product packs are written by the DVE as fp16 at the 2x-mode rate
(~0.55 ns/elem); the PE reads their HIGH BYTES through a stride-2 fp8e5
view (fp16 truncation == e5m2) and consumes PAIRS of packs per fp8
DoubleRow matmul with e4m3 coefficients.  Rotations are pre-scaled by 1/4
on the host (exact in bf16) so products land at p/16, matching the x16
coefficient scaling; everything accumulates into one f32 PSUM group
(2 banks), evicted by Act with bias=tv.  CCOMP compensates the e5m2
truncation bias.

Product groups are ordered so the earliest DVE work needs only the first
rotation slots; input DMAs are split across both HWDGE queues (sync +
scalar) in consumption order.

Sharding: data-parallel over batch B across the 8 cores (1024 rows each);
coefficients replicated.
"""

import os
import sys

sys.path.insert(0, "/opt/trn_rl_repo")

import numpy as np
import ml_dtypes

E, B, D = 128, 8192, 128
NCORES = 8
BLOC = B // NCORES          # 1024 batch rows per core
BT = 512                    # matmul free-dim tile (one PSUM bank)
NSLOT = 16                  # rotation slots: 0..7 then 8,16,...,64
ROTVALS = tuple(range(8)) + tuple(range(8, 65, 8))
NWARM = 5
NPAIR = 32                  # 64 off-diag packs -> 32 DoubleRow pairs
CCOMP = 1.06                # e5m2 truncation compensation (host-tuned)

# product groups: (name, in0 slot range [lo,hi), in1 slot s1).
# pack = r_a (.) r_b with a = ROTVALS[lo+i], b = ROTVALS[s1]; covers the
# wrapped diagonal (b-a) mod 128.  Ordered so early groups need only the
# first slots (g0 runs after slots 0,1 have landed).
GROUPS = [
    ("g0", 1, 2, 0),    # diag 1
    ("g1", 2, 4, 0),    # diag 2,3
    ("g2", 4, 8, 0),    # diag 4..7
    ("g3", 0, 1, 8),    # diag 8
    ("g4", 0, 8, 9),    # diag 16..9
    ("g5", 0, 8, 10),   # diag 24..17
    ("g6", 0, 8, 11),   # diag 32..25
    ("g7", 0, 8, 12),   # diag 40..33
    ("g8", 0, 8, 13),   # diag 48..41
    ("g9", 0, 8, 14),   # diag 56..49
    ("gA", 0, 8, 15),   # diag 64..57
]

PAIRS_AB = []   # (a, b) rotation values per off-diag pack position
for _name, _lo, _hi, _s1 in GROUPS:
    for _i in range(_hi - _lo):
        PAIRS_AB.append((ROTVALS[_lo + _i], ROTVALS[_s1]))
assert len(PAIRS_AB) == 2 * NPAIR
_djs = sorted(min((b - a) % 128, (a - b) % 128) for a, b in PAIRS_AB)
assert _djs == list(range(1, 65)), _djs

bf16 = ml_dtypes.bfloat16
f8e4m3 = ml_dtypes.float8_e4m3

_STATE: dict = {}


def _build_module():
    import concourse.bacc as bacc
    import concourse.tile as tile
    import concourse.mybir as mybir
    from contextlib import ExitStack

    nc = bacc.Bacc("TRN2", target_bir_lowering=False, debug=False)

    xr_d = nc.dram_tensor("xrot", [D, NSLOT * BLOC], mybir.dt.bfloat16,
                          kind="ExternalInput")
    # cwbt: [linear(E) | diag(E) | tv as 2 bf16 halves] per partition row
    cwbt_d = nc.dram_tensor("cwbt", [D, 2 * E + 2], mybir.dt.bfloat16,
                            kind="ExternalInput")
    cwf_d = nc.dram_tensor("cwf", [D, NPAIR * 2 * E], mybir.dt.float8e4,
                           kind="ExternalInput")
    out_d = nc.dram_tensor("out", [E, BLOC], mybir.dt.float32,
                           kind="ExternalOutput")

    f32 = mybir.dt.float32
    b16 = mybir.dt.bfloat16
    f16 = mybir.dt.float16
    f8e5 = mybir.dt.float8e5
    Ident = mybir.ActivationFunctionType.Identity
    DR = mybir.MatmulPerfMode.DoubleRow

    with tile.TileContext(nc) as tc, ExitStack() as ctx:
        const_pool = ctx.enter_context(tc.tile_pool(name="const", bufs=1))
        psum_pool = ctx.enter_context(tc.tile_pool(name="acc", bufs=2, space="PSUM"))

        # PE warmup on a GpSimd-memset tile: no DMA dependency, trips the
        # clock gate during the DMA prologue.
        WU = const_pool.tile([D, BT], b16, tag="warm")
        nc.gpsimd.memset(WU[:, :], 0)
        PSW = psum_pool.tile([E, BT], f32, tag="psw", name="psw")
        for _ in range(NWARM):
            nc.tensor.matmul(PSW[:, :], WU[:, 0:E], WU[:, :],
                             start=True, stop=True, skip_group_check=True)

        ROTS = const_pool.tile([D, NSLOT * BLOC], b16, tag="rots")
        R3 = ROTS[:, :].rearrange("p (s b) -> p s b", s=NSLOT)
        H = const_pool.tile([D, 64 * BLOC], f16, tag="H")
        H3 = H[:, :].rearrange("p (k b) -> p k b", k=64)
        X2 = const_pool.tile([D, BLOC], b16, tag="x2")
        CWBT = const_pool.tile([D, 2 * E + 2], b16, tag="cwbt")
        TV = CWBT[:, 2 * E:2 * E + 2].bitcast(f32)      # [128, 1] f32 bias
        CWF = const_pool.tile([D, NPAIR * 2 * E], mybir.dt.float8e4, tag="cwf")
        CWF4 = CWF[:, :].rearrange("p (t two e) -> p t two e", t=NPAIR, two=2)
        OT = const_pool.tile([E, BLOC], f32, tag="ot")

        # stride-2 fp8e5 view of H high bytes: [D, pack, col, (lo,hi)] -> hi
        He5 = H[:, :].bitcast(f8e5)
        HV = He5.rearrange("p (k b s) -> p k b s", k=64, s=2)[:, :, :, 1:2]

        def dma_rots(eng, lo, hi):
            eng.dma_start(ROTS[:, lo * BLOC:hi * BLOC],
                          xr_d.ap()[:, lo * BLOC:hi * BLOC])

        # DMA schedule, consumption order, both HWDGE queues.
        c0 = 8 * 2 * E          # cwf pairs 0..7
        c1 = 16 * 2 * E         # cwf pairs 8..15
        # sync queue: consts, a-slots (even), big cwf tail
        nc.sync.dma_start(CWBT[:, :], cwbt_d.ap())
        dma_rots(nc.sync, 0, 1)
        nc.sync.dma_start(CWF[:, 0:c0], cwf_d.ap()[:, 0:c0])
        dma_rots(nc.sync, 2, 3)
        dma_rots(nc.sync, 4, 6)
        nc.sync.dma_start(CWF[:, c1:], cwf_d.ap()[:, c1:])
        # scalar queue: odd a-slots, b-slots in consumption order
        dma_rots(nc.scalar, 1, 2)
        dma_rots(nc.scalar, 3, 4)
        dma_rots(nc.scalar, 6, 8)
        dma_rots(nc.scalar, 8, 10)
        nc.scalar.dma_start(CWF[:, c0:c1], cwf_d.ap()[:, c0:c1])
        dma_rots(nc.scalar, 10, 12)
        dma_rots(nc.scalar, 12, 14)
        dma_rots(nc.scalar, 14, 16)

        # x^2 on Act from slot 0 (after its DMA-issue burst)
        nc.scalar.square(X2[:, :], ROTS[:, 0:BLOC])

        PS = psum_pool.tile([E, BLOC], f32, tag="ps", name="ps")

        # linear pack (rhs = x/4) opens both banks
        for bt in range(2):
            nc.tensor.matmul(PS[:, bt * BT:(bt + 1) * BT], CWBT[:, 0:E],
                             ROTS[:, bt * BT:(bt + 1) * BT],
                             start=True, stop=False)

        pos = 0

        def emit_pairs_until(limit):
            nonlocal pos
            while pos + 2 <= limit:
                t = pos // 2
                for h in range(2):
                    rhs = HV[:, 2 * t:2 * t + 2,
                             h * BT:(h + 1) * BT, :].rearrange(
                                 "p two b one -> p two (b one)")
                    nc.tensor.matmul(PS[:, h * BT:(h + 1) * BT],
                                     CWF4[:, t], rhs,
                                     start=False, stop=(t == NPAIR - 1),
                                     perf_mode=DR)
                pos += 2
                if pos == 16:
                    # x^2 pack mid-chain so it doesn't head-block the queue
                    for bt in range(2):
                        nc.tensor.matmul(PS[:, bt * BT:(bt + 1) * BT],
                                         CWBT[:, E:2 * E],
                                         X2[:, bt * BT:(bt + 1) * BT],
                                         start=False, stop=False)

        gpos = 0
        for name, lo, hi, s1 in GROUPS:
            w = hi - lo
            nc.vector.tensor_mul(
                H3[:, gpos:gpos + w, :],
                R3[:, lo:hi, :],
                R3[:, s1:s1 + 1, :].broadcast_to((D, w, BLOC)),
            )
            gpos += w
            emit_pairs_until(gpos)
        assert pos == 64 and gpos == 64

        # eviction with bias=tv, then output DMA on both queues
        nc.scalar.activation(OT[:, 0:BT], PS[:, 0:BT], Ident, bias=TV[:, 0:1])
        nc.sync.dma_start(out_d.ap()[:, 0:BT], OT[:, 0:BT])
        nc.scalar.activation(OT[:, BT:BLOC], PS[:, BT:BLOC], Ident,
                             bias=TV[:, 0:1])
        nc.scalar.dma_start(out_d.ap()[:, BT:BLOC], OT[:, BT:BLOC])

    nc.compile()
    return nc


def _host_precompute(Centroids: np.ndarray, Sigmas: np.ndarray):
    Sinv = np.linalg.inv(Sigmas.astype(np.float64))
    A = 0.5 * (Sinv + np.swapaxes(Sinv, 1, 2))          # [E, D, D]
    c = Centroids[:, 0, :].astype(np.float64)           # [E, D]
    Ac = np.einsum("edk,ek->ed", A, c)

    idx = np.arange(D)
    cwb = np.zeros((D, 2 * E + 2), np.float32)
    cwb[:, 0:E] = (-2.0 * 4.0 * Ac.T)                   # rhs is x/4
    cwb[:, E:2 * E] = 16.0 * A[:, idx, idx].T           # rhs is (x/4)^2
    cwbt_host = np.ascontiguousarray(cwb).astype(bf16)
    tv = np.einsum("ed,ed->e", Ac, c).astype(np.float32)
    cwbt_host[:, 2 * E:2 * E + 2] = np.ascontiguousarray(
        tv[:, None]).view(bf16).reshape(D, 2)

    cwf = np.zeros((D, 64, E), np.float32)
    for k, (a, b) in enumerate(PAIRS_AB):
        dj = (b - a) % 128
        s = 1.0 if dj == 64 else 2.0
        cwf[:, k, :] = (s * 16.0 * CCOMP) * A[:, (idx + a) % D,
                                              (idx + b) % D].T
    cwf_host = np.ascontiguousarray(cwf.reshape(D, NPAIR * 2 * E)).astype(f8e4m3)
    return cwbt_host, cwf_host


def _get_nc():
    if "nc" not in _STATE:
        os.environ.setdefault("JAX_COMPILATION_CACHE_DIR", "/root/.jax_cache")
        _STATE["nc"] = _build_module()
    return _STATE["nc"]


def _make_in_maps(x, Centroids, Sigmas):
    cwbt_host, cwf_host = _host_precompute(
        np.asarray(Centroids, np.float32), np.asarray(Sigmas, np.float32)
    )
    # rotations pre-scaled by 1/4 (exact in bf16): products land at p/16,
    # matching the x16 coefficient scaling -> single-PSUM accumulation
    xT = np.ascontiguousarray(np.asarray(x, np.float32).T * 0.25).astype(bf16)
    in_maps = []
    for cidx in range(NCORES):
        xTs = np.ascontiguousarray(xT[:, cidx * BLOC:(cidx + 1) * BLOC])
        xrot = np.concatenate(
            [np.roll(xTs, -r, axis=0) for r in ROTVALS], axis=1)   # [D, 16*BLOC]
        in_maps.append({
            "xrot": np.ascontiguousarray(xrot),
            "cwbt": cwbt_host,
            "cwf": cwf_host,
        })
    return in_maps


def _run_device(in_maps, trace=False):
    from concourse import bass_utils

    nc = _get_nc()
    return bass_utils.run_bass_kernel_spmd(
        nc, in_maps, core_ids=list(range(NCORES)), trace=trace
    )


def kernel(x, Centroids, Sigmas):
    in_maps = _make_in_maps(x, Centroids, Sigmas)
    res = _run_device(in_maps)
    outT = np.concatenate([res.results[c]["out"] for c in range(NCORES)], axis=1)
    return np.ascontiguousarray(outT.T).astype(np.float32)


# revision 6
# speedup vs baseline: 1.6032x; 1.0333x over previous
"""Trainium2 Bass kernel for the DEN (Mahalanobis distance) layer.

Computes out[b, e] = (x_b - c_e)^T Sigma_e^{-1} (x_b - c_e) for
x [8192, 128], Centroids [128, 1, 128], Sigmas [128, 128, 128].

Strategy (v6: fp16-highbyte fp8 DoubleRow)
------------------------------------------
Wrapped-diagonal decomposition (66 coefficient packs).  The 64 off-diagonal
product packs are written by the DVE as fp16 at the 2x-mode rate
(~0.55 ns/elem); the PE reads their HIGH BYTES through a stride-2 fp8e5
view (fp16 truncation == e5m2) and consumes PAIRS of packs per fp8
DoubleRow matmul with e4m3 coefficients.  Rotations are pre-scaled by 1/4
on the host (exact in bf16) so products land at p/16, matching the x16
coefficient scaling; everything accumulates into one f32 PSUM group
(2 banks), evicted by Act with bias=tv.  CCOMP compensates the e5m2
truncation bias.

Product groups are ordered so the earliest DVE work needs only the first
rotation slots; input DMAs are split across both HWDGE queues (sync +
scalar) in consumption order.

Sharding: data-parallel over batch B across the 8 cores (1024 rows each);
coefficients replicated.
"""

import os
import sys

sys.path.insert(0, "/opt/trn_rl_repo")

import numpy as np
import ml_dtypes

E, B, D = 128, 8192, 128
NCORES = 8
BLOC = B // NCORES          # 1024 batch rows per core
BT = 512                    # matmul free-dim tile (one PSUM bank)
NSLOT = 16                  # rotation slots: 0..7 then 8,16,...,64
ROTVALS = tuple(range(8)) + tuple(range(8, 65, 8))
NWARM = 5
NPAIR = 32                  # 64 off-diag packs -> 32 DoubleRow pairs
CCOMP = 1.06                # e5m2 truncation compensation (host-tuned)

# product groups: (name, in0 slot range [lo,hi), in1 slot s1).
# pack = r_a (.) r_b with a = ROTVALS[lo+i], b = ROTVALS[s1]; covers the
# wrapped diagonal (b-a) mod 128.  Ordered so early groups need only the
# first slots (g0 runs after slots 0,1 have landed).
GROUPS = [
    ("g0", 1, 2, 0),    # diag 1
    ("g1", 2, 4, 0),    # diag 2,3
    ("g2", 4, 8, 0),    # diag 4..7
    ("g3", 0, 1, 8),    # diag 8
    ("g4", 0, 8, 9),    # diag 16..9
    ("g5", 0, 8, 10),   # diag 24..17
    ("g6", 0, 8, 11),   # diag 32..25
    ("g7", 0, 8, 12),   # diag 40..33
    ("g8", 0, 8, 13),   # diag 48..41
    ("g9", 0, 8, 14),   # diag 56..49
    ("gA", 0, 8, 15),   # diag 64..57
]

PAIRS_AB = []   # (a, b) rotation values per off-diag pack position
for _name, _lo, _hi, _s1 in GROUPS:
    for _i in range(_hi - _lo):
        PAIRS_AB.append((ROTVALS[_lo + _i], ROTVALS[_s1]))
assert len(PAIRS_AB) == 2 * NPAIR
_djs = sorted(min((b - a) % 128, (a - b) % 128) for a, b in PAIRS_AB)
assert _djs == list(range(1, 65)), _djs

bf16 = ml_dtypes.bfloat16
f8e4m3 = ml_dtypes.float8_e4m3

_STATE: dict = {}


def _build_module():
    import concourse.bacc as bacc
    import concourse.tile as tile
    import concourse.mybir as mybir
    from contextlib import ExitStack

    nc = bacc.Bacc("TRN2", target_bir_lowering=False, debug=False)

    xr_d = nc.dram_tensor("xrot", [D, NSLOT * BLOC], mybir.dt.bfloat16,
                          kind="ExternalInput")
    # cwbt: [linear(E) | diag(E) | tv as 2 bf16 halves] per partition row
    cwbt_d = nc.dram_tensor("cwbt", [D, 2 * E + 2], mybir.dt.bfloat16,
                            kind="ExternalInput")
    cwf_d = nc.dram_tensor("cwf", [D, NPAIR * 2 * E], mybir.dt.float8e4,
                           kind="ExternalInput")
    out_d = nc.dram_tensor("out", [E, BLOC], mybir.dt.float32,
                           kind="ExternalOutput")

    f32 = mybir.dt.float32
    b16 = mybir.dt.bfloat16
    f16 = mybir.dt.float16
    f8e5 = mybir.dt.float8e5
    Ident = mybir.ActivationFunctionType.Identity
    DR = mybir.MatmulPerfMode.DoubleRow

    with tile.TileContext(nc) as tc, ExitStack() as ctx:
        const_pool = ctx.enter_context(tc.tile_pool(name="const", bufs=1))
        psum_pool = ctx.enter_context(tc.tile_pool(name="acc", bufs=2, space="PSUM"))

        # PE warmup on a GpSimd-memset tile: no DMA dependency, trips the
        # clock gate during the DMA prologue.
        WU = const_pool.tile([D, BT], b16, tag="warm")
        nc.gpsimd.memset(WU[:, :], 0)
        PSW = psum_pool.tile([E, BT], f32, tag="psw", name="psw")
        for _ in range(NWARM):
            nc.tensor.matmul(PSW[:, :], WU[:, 0:E], WU[:, :],
                             start=True, stop=True, skip_group_check=True)

        ROTS = const_pool.tile([D, NSLOT * BLOC], b16, tag="rots")
        R3 = ROTS[:, :].rearrange("p (s b) -> p s b", s=NSLOT)
        H = const_pool.tile([D, 64 * BLOC], f16, tag="H")
        H3 = H[:, :].rearrange("p (k b) -> p k b", k=64)
        X2 = const_pool.tile([D, BLOC], b16, tag="x2")
        CWBT = const_pool.tile([D, 2 * E + 2], b16, tag="cwbt")
        TV = CWBT[:, 2 * E:2 * E + 2].bitcast(f32)      # [128, 1] f32 bias
        CWF = const_pool.tile([D, NPAIR * 2 * E], mybir.dt.float8e4, tag="cwf")
        CWF4 = CWF[:, :].rearrange("p (t two e) -> p t two e", t=NPAIR, two=2)
        OT = const_pool.tile([E, BLOC], f32, tag="ot")

        # stride-2 fp8e5 view of H high bytes: [D, pack, col, (lo,hi)] -> hi
        He5 = H[:, :].bitcast(f8e5)
        HV = He5.rearrange("p (k b s) -> p k b s", k=64, s=2)[:, :, :, 1:2]

        def dma_rots(eng, lo, hi):
            eng.dma_start(ROTS[:, lo * BLOC:hi * BLOC],
                          xr_d.ap()[:, lo * BLOC:hi * BLOC])

        # DMA schedule: small chunks, rot-first, issued upfront on both
        # HWDGE queues so transfers overlap across instructions.
        c0 = 8 * 2 * E          # cwf pairs 0..7
        c1 = 16 * 2 * E         # cwf pairs 8..15
        dma_rots(nc.sync, 0, 1)
        dma_rots(nc.scalar, 1, 2)
        dma_rots(nc.sync, 2, 3)
        dma_rots(nc.scalar, 3, 4)
        nc.sync.dma_start(CWF[:, 0:c0], cwf_d.ap()[:, 0:c0])
        dma_rots(nc.scalar, 4, 6)
        dma_rots(nc.sync, 6, 8)
        nc.sync.dma_start(CWBT[:, :], cwbt_d.ap())
        dma_rots(nc.scalar, 8, 10)
        nc.sync.dma_start(CWF[:, c0:c1], cwf_d.ap()[:, c0:c1])
        dma_rots(nc.scalar, 10, 12)
        nc.sync.dma_start(CWF[:, c1:], cwf_d.ap()[:, c1:])
        dma_rots(nc.scalar, 12, 14)
        dma_rots(nc.scalar, 14, 16)

        # x^2 on the otherwise-idle GpSimd (keeps the Act queue free)
        nc.gpsimd.tensor_mul(X2[:, :], ROTS[:, 0:BLOC], ROTS[:, 0:BLOC])

        PS = psum_pool.tile([E, BLOC], f32, tag="ps", name="ps")

        pos = 0

        def emit_pairs_until(limit):
            nonlocal pos
            while pos + 2 <= limit:
                t = pos // 2
                # tail split: last 4 pairs emit all h0 first, then all h1,
                # so bank 0 closes early and eviction overlaps the h1 tail
                halves = (0,) if t >= NPAIR - 4 else (0, 1)
                for h in halves:
                    rhs = HV[:, 2 * t:2 * t + 2,
                             h * BT:(h + 1) * BT, :].rearrange(
                                 "p two b one -> p two (b one)")
                    nc.tensor.matmul(PS[:, h * BT:(h + 1) * BT],
                                     CWF4[:, t], rhs,
                                     start=(pos == 0), stop=(t == NPAIR - 1),
                                     perf_mode=DR)
                pos += 2
                if pos == 8:
                    # linear pack (rhs = x/4) mid-chain
                    for bt in range(2):
                        nc.tensor.matmul(PS[:, bt * BT:(bt + 1) * BT],
                                         CWBT[:, 0:E],
                                         ROTS[:, bt * BT:(bt + 1) * BT],
                                         start=False, stop=False)
                if pos == 16:
                    # x^2 pack mid-chain
                    for bt in range(2):
                        nc.tensor.matmul(PS[:, bt * BT:(bt + 1) * BT],
                                         CWBT[:, E:2 * E],
                                         X2[:, bt * BT:(bt + 1) * BT],
                                         start=False, stop=False)

        gpos = 0
        for name, lo, hi, s1 in GROUPS:
            w = hi - lo
            nc.vector.tensor_mul(
                H3[:, gpos:gpos + w, :],
                R3[:, lo:hi, :],
                R3[:, s1:s1 + 1, :].broadcast_to((D, w, BLOC)),
            )
            gpos += w
            emit_pairs_until(gpos)
        assert pos == 64 and gpos == 64

        # h1 of the last 4 pairs (bank 1 tail)
        for t in range(NPAIR - 4, NPAIR):
            rhs = HV[:, 2 * t:2 * t + 2, BT:2 * BT, :].rearrange(
                "p two b one -> p two (b one)")
            nc.tensor.matmul(PS[:, BT:BLOC], CWF4[:, t], rhs,
                             start=False, stop=(t == NPAIR - 1),
                             perf_mode=DR)

        # bank 0 evicts on Act (overlapping the h1 tail), bank 1 on DVE
        nc.scalar.activation(OT[:, 0:BT], PS[:, 0:BT], Ident, bias=TV[:, 0:1])
        nc.sync.dma_start(out_d.ap()[:, 0:BT], OT[:, 0:BT])
        nc.vector.tensor_scalar_add(OT[:, BT:BLOC], PS[:, BT:BLOC], TV[:, 0:1])
        nc.scalar.dma_start(out_d.ap()[:, BT:BLOC], OT[:, BT:BLOC])

    nc.compile()
    return nc


def _host_precompute(Centroids: np.ndarray, Sigmas: np.ndarray):
    Sinv = np.linalg.inv(Sigmas.astype(np.float64))
    A = 0.5 * (Sinv + np.swapaxes(Sinv, 1, 2))          # [E, D, D]
    c = Centroids[:, 0, :].astype(np.float64)           # [E, D]
    Ac = np.einsum("edk,ek->ed", A, c)

    idx = np.arange(D)
    cwb = np.zeros((D, 2 * E + 2), np.float32)
    cwb[:, 0:E] = (-2.0 * 4.0 * Ac.T)                   # rhs is x/4
    cwb[:, E:2 * E] = 16.0 * A[:, idx, idx].T           # rhs is (x/4)^2
    cwbt_host = np.ascontiguousarray(cwb).astype(bf16)
    tv = np.einsum("ed,ed->e", Ac, c).astype(np.float32)
    cwbt_host[:, 2 * E:2 * E + 2] = np.ascontiguousarray(
        tv[:, None]).view(bf16).reshape(D, 2)

    cwf = np.zeros((D, 64, E), np.float32)
    for k, (a, b) in enumerate(PAIRS_AB):
        dj = (b - a) % 128
        s = 1.0 if dj == 64 else 2.0
        cwf[:, k, :] = (s * 16.0 * CCOMP) * A[:, (idx + a) % D,
                                              (idx + b) % D].T
    cwf_host = np.ascontiguousarray(cwf.reshape(D, NPAIR * 2 * E)).astype(f8e4m3)
    return cwbt_host, cwf_host


def _get_nc():
    if "nc" not in _STATE:
        os.environ.setdefault("JAX_COMPILATION_CACHE_DIR", "/root/.jax_cache")
        _STATE["nc"] = _build_module()
    return _STATE["nc"]


def _make_in_maps(x, Centroids, Sigmas):
    cwbt_host, cwf_host = _host_precompute(
        np.asarray(Centroids, np.float32), np.asarray(Sigmas, np.float32)
    )
    # rotations pre-scaled by 1/4 (exact in bf16): products land at p/16,
    # matching the x16 coefficient scaling -> single-PSUM accumulation
    xT = np.ascontiguousarray(np.asarray(x, np.float32).T * 0.25).astype(bf16)
    in_maps = []
    for cidx in range(NCORES):
        xTs = np.ascontiguousarray(xT[:, cidx * BLOC:(cidx + 1) * BLOC])
        xrot = np.concatenate(
            [np.roll(xTs, -r, axis=0) for r in ROTVALS], axis=1)   # [D, 16*BLOC]
        in_maps.append({
            "xrot": np.ascontiguousarray(xrot),
            "cwbt": cwbt_host,
            "cwf": cwf_host,
        })
    return in_maps


def _run_device(in_maps, trace=False):
    from concourse import bass_utils

    nc = _get_nc()
    return bass_utils.run_bass_kernel_spmd(
        nc, in_maps, core_ids=list(range(NCORES)), trace=trace
    )


def kernel(x, Centroids, Sigmas):
    in_maps = _make_in_maps(x, Centroids, Sigmas)
    res = _run_device(in_maps)
    outT = np.concatenate([res.results[c]["out"] for c in range(NCORES)], axis=1)
    return np.ascontiguousarray(outT.T).astype(np.float32)
